# revision 1
# baseline (speedup 1.0000x reference)
"""TRN2 Bass kernel for nn_BottleneckAdapter: projection -> down -> LayerNorm ->
SwiGLU -> up, data-parallel over 8 NeuronCores; the final residual add runs on
host (saves 16 MB/core of DMA, the device is PE-bound).

Shapes (hardcoded): B=16, S=2048, C=768, Q=1024, D=64. Tokens = B*S = 32768,
4096 per core. All matmuls run in float32r (TF32-class, full PE rate).

Math folding (host-side, exact fp32 algebra):
 - mean-centering of LayerNorm folded into Wdown:  Wdc = Wdown - colmean(Wdown)
   => the down matmul directly yields c = down - mean_d(down).
 - gamma folded into Wl1/Wl2 columns; beta and bl1/bl2 folded into an extra
   ones-row (K=65) bias row => normed_pre = c * rstd is all the kernel needs.

Schedule: per token macro-tile (512 tokens for the first six, 256 for the
last four so the tail latency chains are short), three stages emitted with a
software pipeline skew so PE never waits on the ACT/DVE LayerNorm chain:
  A(t): load X^T tile, mm1 (8q x 6c MM) + psum->sbuf copies, mm2 (8 MM)
  B(t): LN (square/var-mm/sqrt/recip/mult) + o1/gate MM + silu + gate mult
  C(t): mm5 (2 MM per 128-token subtile) + psum->sbuf copy + store
emission order: A0 A1 B0 A2 B1 C0 A3 B2 C1 ...  PSUM->SBUF copies are split
between DVE and ACT to balance engine load; output stores alternate between
the HWDGE (sync) and SWDGE (gpsimd) queues.
"""
import sys
import os

sys.path.insert(0, "/opt/trn_rl_repo")

import numpy as np

import concourse.bass as bass
import concourse.mybir as mybir
import concourse.tile as tile
from concourse import bacc
from concourse import bass_utils

F32 = mybir.dt.float32
F32R = mybir.dt.float32r

NCORES = 8
B, S, C, Q, D = 16, 2048, 768, 1024, 64
TOK = B * S                 # 32768
TPC = TOK // NCORES         # 4096 tokens per core
CS = C // 128               # 6 c-subtiles
QS = Q // 128               # 8 q-subtiles
EPS = 1e-5

# super-tiles (t0, width); width 1024 -> paired mm1 weight loads
SUPERS = [(0, 1024), (1024, 1024), (2048, 1024),
          (3072, 512), (3584, 256), (3840, 256)]

_CACHE = {}


def _build(tiles=None, reps=1):
    nc = bacc.Bacc("TRN2", target_bir_lowering=False, debug=False,
                   enable_asserts=True, num_devices=NCORES)
    xt = nc.dram_tensor("xt", [C, TPC], F32R, kind="ExternalInput").ap()
    wprojT = nc.dram_tensor("wprojT", [C, Q], F32R, kind="ExternalInput").ap()
    wdc = nc.dram_tensor("wdc", [Q, D], F32R, kind="ExternalInput").ap()
    ones64 = nc.dram_tensor("ones64", [D, D], F32R, kind="ExternalInput").ap()
    w1aug = nc.dram_tensor("w1aug", [D + 1, D], F32R, kind="ExternalInput").ap()
    w2aug = nc.dram_tensor("w2aug", [D + 1, D], F32R, kind="ExternalInput").ap()
    wupT = nc.dram_tensor("wupT", [D, Q], F32R, kind="ExternalInput").ap()
    out = nc.dram_tensor("out", [TPC, Q], F32, kind="ExternalOutput").ap()

    xt_r = xt.rearrange("(o p) t -> p o t", p=128)
    wp_r = wprojT.rearrange("(o p) q -> p o q", p=128)
    wd_r = wdc.rearrange("(o p) d -> p o d", p=128)

    with tile.TileContext(nc) as tc:
        with tc.tile_pool(name="wres", bufs=1) as wres, \
             tc.tile_pool(name="xp", bufs=2) as xp, \
             tc.tile_pool(name="pp", bufs=2) as pp, \
             tc.tile_pool(name="sm", bufs=2) as sm, \
             tc.tile_pool(name="sm4", bufs=4) as sm4, \
             tc.tile_pool(name="op", bufs=4) as op, \
             tc.tile_pool(name="ps1", bufs=4, space="PSUM") as ps1, \
             tc.tile_pool(name="ps2", bufs=1, space="PSUM") as ps2, \
             tc.tile_pool(name="pss", bufs=1, space="PSUM") as pss, \
             tc.tile_pool(name="ps5", bufs=2, space="PSUM") as ps5:

            wp = wres.tile([128, CS, Q], F32R)
            wd = wres.tile([128, QS, D], F32R)
            on64 = wres.tile([D, D], F32R)
            w1 = wres.tile([D + 1, D], F32R)
            w2 = wres.tile([D + 1, D], F32R)
            wu = wres.tile([D, Q], F32R)
            epst = wres.tile([D, 1], F32)

            state = {}
            store_q = [0]
            cp = [0]   # copy round-robin

            def copy_mixed(dst, src, act_every=3):
                cp[0] += 1
                if cp[0] % act_every == 0:
                    nc.scalar.copy(dst, src)
                else:
                    nc.vector.tensor_copy(dst, src)

            def stage_a(si, first=0):
                t0, W = SUPERS[si]
                nch = W // 512 if W >= 512 else 1
                cw = W // nch                      # chunk width (512 or 256)
                xtile = xp.tile([128, CS, W], F32R, tag="xtile")
                for c in range(CS):
                    nc.sync.dma_start(xtile[:, c, :], xt_r[:, c, t0:t0 + W])
                    if first == 1:
                        nc.sync.dma_start(wp[:, c, :], wp_r[:, c, :])
                if first == 1:
                    nc.sync.dma_start(wd[:], wd_r[:])
                    nc.gpsimd.memset(epst[:], EPS)
                elif first == 2:
                    nc.sync.dma_start(on64[:], ones64[:])
                    nc.sync.dma_start(w1[:], w1aug[:])
                    nc.sync.dma_start(w2[:], w2aug[:])
                    nc.sync.dma_start(wu[:], wupT[:])
                ptile = pp.tile([128, QS, W], F32R, tag="ptile")
                for q in range(QS):
                    pbank = []
                    for h in range(nch):
                        pb = ps1.tile([128, cw], F32, tag="p1",
                                      name=f"p1_{q}_{h}")
                        pbank.append(pb)
                    for c in range(CS):
                        for h in range(nch):   # same lhsT across chunks
                            nc.tensor.matmul(
                                pbank[h][:], wp[:, c, q * 128:(q + 1) * 128],
                                xtile[:, c, h * cw:(h + 1) * cw],
                                start=(c == 0), stop=(c == CS - 1))
                    for h in range(nch):
                        copy_mixed(ptile[:, q, h * cw:(h + 1) * cw],
                                   pbank[h][:])
                # mm2 per chunk -> early copy of c to SBUF
                for h in range(nch):
                    p2 = ps2.tile([D, cw], F32, tag="p2")
                    for q in range(QS):
                        nc.tensor.matmul(p2[:], wd[:, q, :],
                                         ptile[:, q, h * cw:(h + 1) * cw],
                                         start=(q == 0), stop=(q == QS - 1))
                    c_sb = sm4.tile([D, cw], F32, tag="c_sb")
                    nc.vector.tensor_copy(c_sb[:], p2[:])
                    state[("c", si, h)] = c_sb

            def stage_b(si, h):
                t0, W = SUPERS[si]
                c_sb = state.pop(("c", si, h))
                cw = c_sb.shape[1]
                csq = sm.tile([D, cw], F32R, tag="csq")
                nc.scalar.activation(csq[:], c_sb[:],
                                     mybir.ActivationFunctionType.Square)
                varp = pss.tile([D, cw], F32, tag="small")
                nc.tensor.matmul(varp[:], on64[:], csq[:], start=True, stop=True)
                s = sm.tile([D, cw], F32, tag="s")
                nc.scalar.activation(s[:], varp[:],
                                     mybir.ActivationFunctionType.Sqrt,
                                     bias=epst[:])
                rstd = sm.tile([D, cw], F32, tag="rstd")
                nc.vector.reciprocal(rstd[:], s[:])
                normed = sm.tile([D + 1, cw], F32R, tag="normed")
                nc.vector.tensor_mul(normed[0:D, :], c_sb[:], rstd[:])
                nc.gpsimd.memset(normed[D:D + 1, :].bitcast(F32), 1.0)
                o1p = pss.tile([D, cw], F32, tag="small")
                nc.tensor.matmul(o1p[:], w1[:], normed[:], start=True, stop=True)
                gatep = pss.tile([D, cw], F32, tag="small")
                nc.tensor.matmul(gatep[:], w2[:], normed[:], start=True, stop=True)
                swish = sm.tile([D, cw], F32, tag="swish")
                nc.scalar.activation(swish[:], o1p[:],
                                     mybir.ActivationFunctionType.Silu)
                actT = sm4.tile([D, cw], F32R, tag="actT")
                nc.vector.tensor_mul(actT[:], gatep[:], swish[:])
                state[("actT", si, h)] = actT

            def stage_c(si, h):
                t0, W = SUPERS[si]
                actT = state.pop(("actT", si, h))
                cw = actT.shape[1]
                base = t0 + h * cw
                for ts in range(cw // 128):
                    r0 = base + ts * 128
                    otile = op.tile([128, Q], F32, tag="otile")
                    for qh in range(2):
                        p5 = ps5.tile([128, 512], F32, tag="p5")
                        nc.tensor.matmul(p5[:],
                                         actT[:, ts * 128:(ts + 1) * 128],
                                         wu[:, qh * 512:(qh + 1) * 512],
                                         start=True, stop=True)
                        if si >= 4:   # tail: keep copies on the fast engine
                            nc.vector.tensor_copy(
                                otile[:, qh * 512:(qh + 1) * 512], p5[:])
                        else:
                            copy_mixed(otile[:, qh * 512:(qh + 1) * 512],
                                       p5[:], act_every=2)
                    eng = nc.sync if store_q[0] % 2 == 0 else nc.gpsimd
                    store_q[0] += 1
                    eng.dma_start(out[r0:r0 + 128, :], otile[:])

            def chunks_of(si):
                t0, W = SUPERS[si]
                return [(si, h) for h in range(W // 512 if W >= 512 else 1)]

            NS = len(SUPERS)
            for rep in range(reps):
                stage_a(0, first=1 if rep == 0 else 0)
                stage_a(1, first=2 if rep == 0 else 0)
                for ch in chunks_of(0):
                    stage_b(*ch)
                for si in range(2, NS):
                    stage_a(si)
                    for ch in chunks_of(si - 1):
                        stage_b(*ch)
                    for ch in chunks_of(si - 2):
                        stage_c(*ch)
                for ch in chunks_of(NS - 1):
                    stage_b(*ch)
                for ch in chunks_of(NS - 2):
                    stage_c(*ch)
                for ch in chunks_of(NS - 1):
                    stage_c(*ch)
    nc.compile()
    return nc


def _prep_shared(Wproj, Wdown, gamma, beta, Wl1, bl1, Wl2, bl2, Wup):
    f32 = np.float32
    wprojT = np.ascontiguousarray(Wproj.T).astype(f32, copy=False)
    wdcent = Wdown - Wdown.mean(axis=0, keepdims=True)
    wdc = np.ascontiguousarray(wdcent.T).astype(f32, copy=False)
    ones64 = np.full((D, D), 1.0 / D, dtype=f32)
    w1aug = np.empty((D + 1, D), dtype=f32)
    w1aug[:D] = (Wl1 * gamma[None, :]).T
    w1aug[D] = Wl1 @ beta + bl1
    w2aug = np.empty((D + 1, D), dtype=f32)
    w2aug[:D] = (Wl2 * gamma[None, :]).T
    w2aug[D] = Wl2 @ beta + bl2
    wupT = np.ascontiguousarray(Wup.T).astype(f32, copy=False)
    return dict(wprojT=wprojT, wdc=wdc, ones64=ones64,
                w1aug=w1aug, w2aug=w2aug, wupT=wupT)


def _ref_rows(X_rows, P):
    """numpy reference (up only, no residual) for a few token rows."""
    proj = X_rows @ P["wprojT"]                       # [n, Q]
    c = proj @ P["wdc"]                               # [n, D]
    var = (c * c).mean(axis=1, keepdims=True)
    z = c / np.sqrt(var + EPS)
    zaug = np.concatenate([z, np.ones((z.shape[0], 1), z.dtype)], axis=1)
    o1 = zaug @ P["w1aug"]
    gate = zaug @ P["w2aug"]
    act = o1 / (1.0 + np.exp(-o1)) * gate
    return act @ P["wupT"]


def kernel(clamp3_features, residual, Wproj, Wdown, gamma, beta,
           Wl1, bl1, Wl2, bl2, Wup):
    if "nc" not in _CACHE:
        _CACHE["nc"] = _build()
    nc = _CACHE["nc"]

    f32 = np.float32
    X = np.asarray(clamp3_features, dtype=f32).reshape(TOK, C)
    shared = _prep_shared(np.asarray(Wproj, f32), np.asarray(Wdown, f32),
                          np.asarray(gamma, f32), np.asarray(beta, f32),
                          np.asarray(Wl1, f32), np.asarray(bl1, f32),
                          np.asarray(Wl2, f32), np.asarray(bl2, f32),
                          np.asarray(Wup, f32))

    in_maps = []
    for cid in range(NCORES):
        lo, hi = cid * TPC, (cid + 1) * TPC
        in_maps.append({"xt": np.ascontiguousarray(X[lo:hi].T), **shared})

    # sampled self-check rows (2 per core) to catch transient bad executions
    rng = np.random.default_rng(12345)
    sample = np.sort(rng.choice(TPC, size=2, replace=False))
    Pd = {k: shared[k].astype(np.float64) for k in
          ("wprojT", "wdc", "w1aug", "w2aug", "wupT")}

    for attempt in range(3):
        res = bass_utils.run_bass_kernel_spmd(nc, in_maps,
                                              core_ids=list(range(NCORES)))
        outs = [res.results[cid]["out"] for cid in range(NCORES)]
        ok = True
        for cid in range(NCORES):
            rows = cid * TPC + sample
            ref = _ref_rows(X[rows].astype(np.float64), Pd)
            got = outs[cid][sample].astype(np.float64)
            err = np.abs(got - ref).max() / max(np.abs(ref).max(), 1e-30)
            if not np.isfinite(err) or err > 5e-3:
                ok = False
                break
        if ok:
            break

    up = np.concatenate(outs, axis=0).reshape(B, S, Q)
    return (np.asarray(residual, dtype=f32) + up).astype(np.float32, copy=False)



# revision 2
# speedup vs baseline: 2.3336x; 2.3336x over previous
"""TRN2 Bass kernel for nn_BottleneckAdapter, data-parallel over 8 NeuronCores.

Key algebraic fold (host-side, exact): the reference computes
  projected = X @ Wproj^T ; down = projected @ Wdown^T
and `projected` is used nowhere else, so
  down = X @ (Wdown @ Wproj)^T  =  X @ M,   M = (Wdown @ Wproj)^T  [C=768, D=64].
LayerNorm mean-centering folds into M (subtract per-row mean over D), gamma
folds into Wl1/Wl2, beta/bl1/bl2 fold into per-feature biases (zero for this
problem's inputs -> fast path without bias ops).

Per-core shapes: 4096 tokens, C=768, D=64, Q=1024.  All I/O in fp16 (X, up
output); PSUM accumulates fp32.  The residual add runs on host.

Layout trick: two 512-token chunks are stacked along the 128 partitions
([c_A; c_B], 64 feature rows each) so the LayerNorm + SwiGLU chain runs at
full 128-lane width.  Per-token variance is produced replicated across the
64 feature rows by a block-diagonal ones matmul; o1/gate use block-diagonal
(w1^T, w1^T) weights; the second half's matmuls use partition-offset outputs
(tile_position is derived from out/lhsT base partitions by Bass).

Per pair-tile (1024 tokens): 1 input DMA (1.5 MB), 31 matmuls, 6 ACT/DVE
LayerNorm/SwiGLU ops, 8 PSUM->SBUF cast copies (split ACT/DVE), 2 output
DMAs (1 MB each, SWDGE).  Engine busy/core ~26-32 us each; DMA ~15 MB.
"""
import sys

sys.path.insert(0, "/opt/trn_rl_repo")

import numpy as np

import concourse.bass as bass
import concourse.mybir as mybir
import concourse.tile as tile
from concourse import bacc
from concourse import bass_utils

F32 = mybir.dt.float32
F16 = mybir.dt.float16

NCORES = 8
B, S, C, Q, D = 16, 2048, 768, 1024, 64
TOK = B * S                 # 32768
TPC = TOK // NCORES         # 4096 tokens per core
CS = C // 128               # 6 c-subtiles
CW = 512                    # tokens per stacked half-chunk
PAIR = 2 * CW               # tokens per pair-tile
NPT = TPC // PAIR           # 4 pair-tiles per core
EPS = 1e-5

_CACHE = {}


def _build(reps=1, with_bias=False):
    nc = bacc.Bacc("TRN2", target_bir_lowering=False, debug=False,
                   enable_asserts=True, num_devices=NCORES)
    xt = nc.dram_tensor("xt", [C, TPC], F16, kind="ExternalInput").ap()
    wcc = nc.dram_tensor("wcc", [C, D], F16, kind="ExternalInput").ap()
    onesbd = nc.dram_tensor("onesbd", [128, 128], F16, kind="ExternalInput").ap()
    w1bd = nc.dram_tensor("w1bd", [128, 128], F16, kind="ExternalInput").ap()
    w2bd = nc.dram_tensor("w2bd", [128, 128], F16, kind="ExternalInput").ap()
    wu2 = nc.dram_tensor("wu2", [128, Q], F16, kind="ExternalInput").ap()
    if with_bias:
        b1d = nc.dram_tensor("b1d", [128, 1], F32, kind="ExternalInput").ap()
        b2d = nc.dram_tensor("b2d", [128, 1], F32, kind="ExternalInput").ap()
    out = nc.dram_tensor("out", [TPC, Q], F16, kind="ExternalOutput").ap()

    xt_r = xt.rearrange("(o p) t -> p o t", p=128)
    wcc_r = wcc.rearrange("(o p) d -> p o d", p=128)
    out_r = out.rearrange("(g p) q -> p g q", p=128)   # g: 32 groups of 128 rows

    with tile.TileContext(nc) as tc:
        with tc.tile_pool(name="wres", bufs=1) as wres, \
             tc.tile_pool(name="xp", bufs=3) as xp, \
             tc.tile_pool(name="sq", bufs=2) as sqp, \
             tc.tile_pool(name="sr", bufs=2) as srp, \
             tc.tile_pool(name="sn", bufs=2) as snp, \
             tc.tile_pool(name="ss", bufs=2) as ssp, \
             tc.tile_pool(name="sa", bufs=2) as sap, \
             tc.tile_pool(name="op", bufs=3) as op, \
             tc.tile_pool(name="pc", bufs=1, space="PSUM") as pcp, \
             tc.tile_pool(name="pv", bufs=1, space="PSUM") as pvp, \
             tc.tile_pool(name="po", bufs=1, space="PSUM") as pop, \
             tc.tile_pool(name="pg", bufs=1, space="PSUM") as pgp, \
             tc.tile_pool(name="pu", bufs=2, space="PSUM") as pup:

            wcc_sb = wres.tile([128, CS, D], F16)
            ones_sb = wres.tile([128, 128], F16)
            w1_sb = wres.tile([128, 128], F16)
            w2_sb = wres.tile([128, 128], F16)
            wu_sb = wres.tile([128, Q], F16)
            epst = wres.tile([128, 1], F32)
            if with_bias:
                b1t = wres.tile([128, 1], F32)
                b2t = wres.tile([128, 1], F32)

            cpr = [0]  # copy round-robin between DVE and ACT

            def pair_tile(pt, first=False):
                t0 = pt * PAIR
                xtile = xp.tile([128, CS, PAIR], F16, tag="x")
                nc.sync.dma_start(xtile[:], xt_r[:, :, t0:t0 + PAIR])
                if first:
                    nc.sync.dma_start(wcc_sb[:], wcc_r[:])
                    nc.sync.dma_start(ones_sb[:], onesbd[:])
                    nc.sync.dma_start(w1_sb[:], w1bd[:])
                    nc.sync.dma_start(w2_sb[:], w2bd[:])
                    nc.sync.dma_start(wu_sb[:], wu2[:])
                    nc.gpsimd.memset(epst[:], EPS)
                    if with_bias:
                        nc.sync.dma_start(b1t[:], b1d[:])
                        nc.sync.dma_start(b2t[:], b2d[:])

                # down-proj: c = X @ M, two 512-token halves stacked on
                # partitions (A -> 0:64, B -> 64:128)
                psc = pcp.tile([128, CW], F32, tag="pc")
                for h in range(2):
                    dst = psc[64 * h:64 * h + 64, :]
                    for c in range(CS):
                        nc.tensor.matmul(dst, wcc_sb[:, c, :],
                                         xtile[:, c, h * CW:(h + 1) * CW],
                                         start=(c == 0), stop=(c == CS - 1))

                csq = sqp.tile([128, CW], F16, tag="csq")
                nc.scalar.activation(csq[:], psc[:],
                                     mybir.ActivationFunctionType.Square)
                psv = pvp.tile([128, CW], F32, tag="pv")
                nc.tensor.matmul(psv[:], ones_sb[:], csq[:], start=True,
                                 stop=True)
                s = srp.tile([128, CW], F32, tag="s")
                nc.scalar.activation(s[:], psv[:],
                                     mybir.ActivationFunctionType.Sqrt,
                                     bias=epst[:])
                rstd = srp.tile([128, CW], F32, tag="rstd")
                nc.vector.reciprocal(rstd[:], s[:])
                normed = snp.tile([128, CW], F16, tag="normed")
                nc.vector.tensor_mul(normed[:], psc[:], rstd[:])

                pso = pop.tile([128, CW], F32, tag="po")
                nc.tensor.matmul(pso[:], w1_sb[:], normed[:], start=True,
                                 stop=True)
                psg = pgp.tile([128, CW], F32, tag="pg")
                nc.tensor.matmul(psg[:], w2_sb[:], normed[:], start=True,
                                 stop=True)
                swish = ssp.tile([128, CW], F16, tag="swish")
                if with_bias:
                    nc.scalar.activation(swish[:], pso[:],
                                         mybir.ActivationFunctionType.Silu,
                                         bias=b1t[:])
                    gb = ssp.tile([128, CW], F32, tag="gb")
                    nc.scalar.activation(gb[:], psg[:],
                                         mybir.ActivationFunctionType.Identity,
                                         bias=b2t[:])
                    gsrc = gb
                else:
                    nc.scalar.activation(swish[:], pso[:],
                                         mybir.ActivationFunctionType.Silu)
                    gsrc = psg
                actT = sap.tile([128, CW], F16, tag="actT")
                nc.vector.tensor_mul(actT[:], gsrc[:], swish[:])

                # up-proj: out[t, :] = act[t] @ Wup^T, per 128-token subtile
                for h in range(2):
                    ocp = op.tile([128, 4, Q], F16, tag="ocp")
                    for ts in range(CW // 128):
                        psu = pup.tile([128, Q], F32, tag="pu")
                        lhsT = actT[64 * h:64 * h + 64,
                                    ts * 128:(ts + 1) * 128]
                        for qh in range(2):
                            nc.tensor.matmul(
                                psu[:, qh * 512:(qh + 1) * 512], lhsT,
                                wu_sb[64 * h:64 * h + 64,
                                      qh * 512:(qh + 1) * 512],
                                start=True, stop=True)
                        cpr[0] += 1
                        if cpr[0] % 2 == 0:
                            nc.vector.tensor_copy(ocp[:, ts, :], psu[:])
                        else:
                            nc.scalar.copy(ocp[:, ts, :], psu[:])
                    g0 = pt * 8 + 4 * h
                    nc.gpsimd.dma_start(out_r[:, g0:g0 + 4, :], ocp[:])

            for rep in range(reps):
                for pt in range(NPT):
                    pair_tile(pt, first=(rep == 0 and pt == 0))
    nc.compile()
    return nc


def _prep_shared(Wproj, Wdown, gamma, beta, Wl1, bl1, Wl2, bl2, Wup):
    f64, f16, f32 = np.float64, np.float16, np.float32
    M = (Wdown.astype(f64) @ Wproj.astype(f64)).T          # [C, D]
    M = M - M.mean(axis=1, keepdims=True)                  # fold LN centering
    wcc = np.ascontiguousarray(M).astype(f16)
    onesbd = np.zeros((128, 128), dtype=f16)
    onesbd[:D, :D] = 1.0 / D
    onesbd[D:, D:] = 1.0 / D
    w1g = np.ascontiguousarray((Wl1 * gamma[None, :]).T)   # [d, e]
    w2g = np.ascontiguousarray((Wl2 * gamma[None, :]).T)
    w1bd = np.zeros((128, 128), dtype=f16)
    w1bd[:D, :D] = w1g
    w1bd[D:, D:] = w1g
    w2bd = np.zeros((128, 128), dtype=f16)
    w2bd[:D, :D] = w2g
    w2bd[D:, D:] = w2g
    wu2 = np.empty((128, Q), dtype=f16)
    wu2[:D] = Wup.T
    wu2[D:] = Wup.T
    b1 = (Wl1.astype(f64) @ beta.astype(f64) + bl1).astype(f32)
    b2 = (Wl2.astype(f64) @ beta.astype(f64) + bl2).astype(f32)
    shared = dict(wcc=wcc, onesbd=onesbd, w1bd=w1bd, w2bd=w2bd, wu2=wu2)
    with_bias = bool(np.any(b1 != 0) or np.any(b2 != 0))
    if with_bias:
        shared["b1d"] = np.concatenate([b1, b1]).reshape(128, 1)
        shared["b2d"] = np.concatenate([b2, b2]).reshape(128, 1)
    return shared, with_bias


def _ref_rows(X_rows, P):
    """float64 reference (up only, no residual) for a few token rows."""
    c = X_rows @ P["wcc"]                                  # [n, D]
    var = (c * c).mean(axis=1, keepdims=True)
    z = c / np.sqrt(var + EPS)
    o1 = z @ P["w1bd"][:D, :D] + P.get("b1", 0.0)
    gate = z @ P["w2bd"][:D, :D] + P.get("b2", 0.0)
    act = o1 / (1.0 + np.exp(-o1)) * gate
    return act @ P["wu2"][:D]


def kernel(clamp3_features, residual, Wproj, Wdown, gamma, beta,
           Wl1, bl1, Wl2, bl2, Wup):
    f32, f16 = np.float32, np.float16
    X = np.asarray(clamp3_features, dtype=f32).reshape(TOK, C)
    shared, with_bias = _prep_shared(
        np.asarray(Wproj, f32), np.asarray(Wdown, f32),
        np.asarray(gamma, f32), np.asarray(beta, f32),
        np.asarray(Wl1, f32), np.asarray(bl1, f32),
        np.asarray(Wl2, f32), np.asarray(bl2, f32), np.asarray(Wup, f32))

    key = ("nc", with_bias)
    if key not in _CACHE:
        _CACHE[key] = _build(with_bias=with_bias)
    nc = _CACHE[key]

    X16 = X.astype(f16)
    in_maps = []
    for cid in range(NCORES):
        lo, hi = cid * TPC, (cid + 1) * TPC
        in_maps.append({"xt": np.ascontiguousarray(X16[lo:hi].T), **shared})

    # sampled self-check rows (2 per core) to catch transient bad executions
    rng = np.random.default_rng(12345)
    sample = np.sort(rng.choice(TPC, size=2, replace=False))
    Pd = {k: shared[k].astype(np.float64) for k in ("wcc", "w1bd", "w2bd",
                                                    "wu2")}
    if with_bias:
        Pd["b1"] = shared["b1d"][:D, 0].astype(np.float64)
        Pd["b2"] = shared["b2d"][:D, 0].astype(np.float64)

    for attempt in range(3):
        res = bass_utils.run_bass_kernel_spmd(nc, in_maps,
                                              core_ids=list(range(NCORES)))
        outs = [res.results[cid]["out"] for cid in range(NCORES)]
        ok = True
        for cid in range(NCORES):
            rows = cid * TPC + sample
            ref = _ref_rows(X[rows].astype(np.float64), Pd)
            got = outs[cid][sample].astype(np.float64)
            err = np.abs(got - ref).max() / max(np.abs(ref).max(), 1e-30)
            if not np.isfinite(err) or err > 1e-2:
                ok = False
                break
        if ok:
            break

    up = np.concatenate(outs, axis=0).astype(f32).reshape(B, S, Q)
    return (np.asarray(residual, dtype=f32) + up).astype(np.float32,
                                                         copy=False)
